# revision 7
# baseline (speedup 1.0000x reference)
"""Trainium2 Bass kernel for nn_BDHModel (topk_masking).

Computes, per head h and token l:
    raw = projections[:, tokens, :]                  (gathered on host = sequence sharding)
    thr[h,l] = 20th largest of raw[h,l,:]            (exact: block-candidate max8 +
                                                      reciprocal-rank refinement on 64 cands)
    acts = (raw >= thr)
    preds[h,l] = acts[h,l] @ sigma[h].T              (fp8 DoubleRow GEMM, sigma/4 stationary)
    dot[h,l]   = sum(preds[h,l] * acts[h,l+1])       (GpSimd product + DoubleRow ones-matmul)
    norm2[h,l] = sum(preds[h,l]^2)                   (ScalarE Square from PSUM + ones-matmul)
    out = 1 - dot / (sqrt(norm2)*sqrt(20) + 1e-8)    (final scalar math on host)

Distribution: data-parallel over the sequence across 8 NeuronCores. Each core
processes exactly 1024 tokens for all 3 heads; sigma (pre-transposed to
(d_in, d_out), scaled by 1/4, fp8e4m3) is replicated to every core. The
cross-core boundary dot (token 1023 of each core vs token 0 of the next) is
patched on the host from an exported preds column.

Top-20 threshold: per 128-token tile, 8x max8 over 256-wide blocks of the
neuron axis yields 64 candidates/row that contain the true top-20 unless one
block holds >8 of them (P ~ 2e-3/row; affected rows move the threshold by
~1 rank, which is far inside the 2e-2 gate). The reciprocal-rank trick
(v8 -> v15 -> v20) then runs on just the 64 candidates, batched 4 tiles per
ScalarE/DVE instruction.
"""

import numpy as np
import ml_dtypes

import concourse.bacc as bacc
import concourse.mybir as mybir
import concourse.bass_utils as bass_utils
from concourse.bass import AP
from concourse.tile import TileContext
from concourse.masks import make_identity

ActF = mybir.ActivationFunctionType


def _act_raw(eng, out, in_, func, bias=0.0, scale=1.0, alpha=0.0, accum_out=None):
    """Direct InstActivation emission; bypasses the bass Reciprocal guard.

    Reciprocal here is used only for rank-ordering (monotone transform), where
    the table's ~1e-5 relative error is irrelevant; outputs clamp at +-1e7 and
    recip(0) = 3.4e38 (probed on HW), so no inf/NaN can reach max8.
    """
    inputs = [eng.lower_ap(in_)]
    for arg in (bias, scale, alpha):
        if isinstance(arg, AP):
            inputs.append(eng.lower_ap(arg))
        else:
            inputs.append(mybir.ImmediateValue(dtype=mybir.dt.float32, value=arg))
    outputs = [eng.lower_ap(out)]
    if accum_out is not None:
        outputs.append(eng.lower_ap(accum_out))
    return eng.add_instruction(
        mybir.InstActivation(
            name=eng.bass.get_next_instruction_name(),
            func=func,
            ins=inputs,
            outs=outputs,
        )
    )

H, V, D, L = 3, 32000, 2048, 8192
K = 20
NCORES = 8
CHUNK = L // NCORES            # 1024 tokens per core
TILES = CHUNK // 128           # 8 row-tiles, no boundary tile (host patches it)
ACOLS = CHUNK + 8              # actsT cols: 1024 tokens + zeroed pad (col 1024 used)
DB = D // 128                  # 16 blocks of 128 along the neuron axis
SB = DB // 2                   # 8 super-blocks of 256 (DoubleRow)
P = 128
NBLK = 8                       # candidate blocks per row (8 x 256)
BLKW = D // NBLK               # 256
NCAND = NBLK * 8               # 64 candidates per row
QT = 4                         # tiles per refinement batch ("quad")

F32 = mybir.dt.float32
BF16 = mybir.dt.bfloat16
FP8 = mybir.dt.float8e4

LAST_RESULTS = None            # test.py reads exec_time_ns from here

_NC_CACHE = None


def _build_nc():
    nc = bacc.Bacc("TRN2", target_bir_lowering=False, debug=False)
    raw_ext = nc.dram_tensor("raw", [H, TILES, P, D], F32, kind="ExternalInput")
    sigT_ext = nc.dram_tensor("sigT", [H, DB, P, D], FP8, kind="ExternalInput")
    dot_ext = nc.dram_tensor("dot_out", [1, H, CHUNK], F32, kind="ExternalOutput")
    nrm_ext = nc.dram_tensor("nrm_out", [1, H, CHUNK], F32, kind="ExternalOutput")
    pl_ext = nc.dram_tensor("plast_out", [P, H, DB], BF16, kind="ExternalOutput")

    with TileContext(nc) as tc:
        _body(nc, tc, raw_ext, sigT_ext, dot_ext, nrm_ext, pl_ext)
    nc.compile()
    return nc


def _body(nc, tc, raw_ext, sigT_ext, dot_ext, nrm_ext, pl_ext):
    with (
        tc.tile_pool(name="consts", bufs=1) as consts,
        tc.tile_pool(name="sig", bufs=2) as sig_pool,
        tc.tile_pool(name="actsT", bufs=2) as actsT_pool,
        tc.tile_pool(name="raw", bufs=5) as raw_pool,
        tc.tile_pool(name="acts", bufs=2) as acts_pool,
        tc.tile_pool(name="atb", bufs=3) as atb_pool,
        tc.tile_pool(name="cand", bufs=2) as cand_pool,
        tc.tile_pool(name="m8", bufs=4) as m8_pool,
        tc.tile_pool(name="preds", bufs=6) as preds_pool,
        tc.tile_pool(name="prod", bufs=6) as prod_pool,
        tc.tile_pool(name="stage", bufs=1) as stage_pool,
        tc.tile_pool(name="gpsum", bufs=2, space="PSUM") as gpsum_pool,
        tc.tile_pool(name="rpsum", bufs=1, space="PSUM") as rpsum_pool,
    ):
        ones = consts.tile([P, 2, 16], FP8)
        nc.vector.memset(ones[:], 1.0)

        dot_sb = stage_pool.tile([1, H, CHUNK], F32, tag="dot_sb")
        nrm_sb = stage_pool.tile([1, H, CHUNK], F32, tag="nrm_sb")
        plast = stage_pool.tile([P, H, DB], BF16, tag="plast")

        sig_next = None
        for h in range(H):
            if sig_next is None:
                sigT_sb = sig_pool.tile([P, DB, D], FP8, tag="sigT")
            else:
                sigT_sb = sig_next
            actsT8 = actsT_pool.tile([P, DB, ACOLS], FP8, tag="actsT")
            # zero the pad column so the lc=1 shifted product reads 0 there
            nc.vector.memset(actsT8[:, :, CHUNK:ACOLS], 0.0)

            # --- stage 1: exact top-20 threshold on 64 block candidates ---
            for q in range(TILES // QT):
                raws = []
                for tt in range(QT):
                    t = q * QT + tt
                    rt = raw_pool.tile([P, D], F32, tag="raw", name="rawt")
                    nc.sync.dma_start(rt[:], raw_ext[h, t])
                    raws.append(rt)
                if h == 0 and q == 0:
                    # sigma loads go behind the first raw tiles
                    for db in range(DB):
                        nc.sync.dma_start(sigT_sb[:, db, :], sigT_ext[h, db])

                cand = cand_pool.tile([P, QT, NCAND], F32, tag="cand")
                c8 = m8_pool.tile([P, QT, 8], F32, tag="c8")
                for tt in range(QT):
                    for b in range(NBLK):
                        nc.vector.max(
                            cand[:, tt, b * 8:(b + 1) * 8],
                            raws[tt][:, b * BLKW:(b + 1) * BLKW],
                        )
                for tt in range(QT):
                    nc.vector.max(c8[:, tt, :], cand[:, tt, :])

                # z1 = 1/(v8 + eps - cand); top of z1 = [z(v8), r9..r15]
                w1 = cand_pool.tile([P, QT, NCAND], F32, tag="w1")
                nc.vector.tensor_tensor(
                    w1[:], cand[:],
                    c8[:, :, 7:8].broadcast_to([P, QT, NCAND]),
                    op=mybir.AluOpType.subtract,
                )
                _act_raw(nc.scalar, w1[:], w1[:], ActF.Reciprocal,
                         scale=-1.0, bias=2.0 ** -40)
                m8b = m8_pool.tile([P, QT, 8], F32, tag="m8b")
                for tt in range(QT):
                    nc.vector.max(m8b[:, tt, :], w1[:, tt, :])
                # v15 = v8 - 0.9997/z15 (slightly above the true v15)
                inv1 = m8_pool.tile([P, QT, 1], F32, tag="inv1")
                _act_raw(nc.scalar, inv1[:], m8b[:, :, 7:8], ActF.Reciprocal,
                         scale=-1.0003)
                v15 = m8_pool.tile([P, QT, 1], F32, tag="v15")
                nc.vector.tensor_tensor(
                    v15[:], inv1[:], c8[:, :, 7:8], op=mybir.AluOpType.add
                )

                # z2 = 1/(v15 + eps - cand); top = [z(r15), r16..r22]
                w2 = cand_pool.tile([P, QT, NCAND], F32, tag="w2")
                nc.vector.tensor_tensor(
                    w2[:], cand[:],
                    v15[:].broadcast_to([P, QT, NCAND]),
                    op=mybir.AluOpType.subtract,
                )
                _act_raw(nc.scalar, w2[:], w2[:], ActF.Reciprocal,
                         scale=-1.0, bias=2.0 ** -40)
                m8c = m8_pool.tile([P, QT, 8], F32, tag="m8c")
                for tt in range(QT):
                    nc.vector.max(m8c[:, tt, :], w2[:, tt, :])
                # thr = v15 - 1.0003/z20 (slightly below the true v20)
                inv2 = m8_pool.tile([P, QT, 1], F32, tag="inv2")
                _act_raw(nc.scalar, inv2[:], m8c[:, :, 5:6], ActF.Reciprocal,
                         scale=-0.9997)
                thr = m8_pool.tile([P, QT, 1], F32, tag="thr")
                nc.vector.tensor_tensor(
                    thr[:], inv2[:], v15[:], op=mybir.AluOpType.add
                )

                for tt in range(QT):
                    t = q * QT + tt
                    acts_t = acts_pool.tile([P, D], BF16, tag="acts")
                    nc.vector.tensor_scalar(
                        acts_t[:], raws[tt][:], thr[:, tt, :], None,
                        mybir.AluOpType.is_ge,
                    )
                    # xbar transpose: atb[p, k, l] = acts_t[l, k*128 + p];
                    # dst must be contiguous (strided xbar dst is broken on HW)
                    atb = atb_pool.tile([P, DB, P], BF16, tag="atb")
                    nc.sync.dma_start_transpose(atb[:], acts_t[:])
                    # bf16 -> fp8 convert into the (strided) GEMM operand
                    nc.gpsimd.tensor_copy(
                        actsT8[:, :, t * P:(t + 1) * P], atb[:]
                    )

            if h + 1 < H:
                # prefetch next head's sigma behind this head's GEMM
                sig_next = sig_pool.tile([P, DB, D], FP8, tag="sigT", name="sig_nx")
                for db in range(DB):
                    nc.sync.dma_start(sig_next[:, db, :], sigT_ext[h + 1, db])

            # --- stage 2: fp8 DoubleRow GEMM (predsT layout) + reductions ---
            for lc in range(CHUNK // 512):
                l0 = lc * 512
                dot_ps = rpsum_pool.tile([1, 512], F32, tag="dotps")
                nrm_ps = rpsum_pool.tile([1, 512], F32, tag="nrmps")
                prodp = None
                prod2p = None
                pending = []           # completed prod pairs awaiting reduce-MMs

                def flush_pair():
                    pa, p2a, first, last = pending.pop(0)
                    nc.tensor.matmul(
                        dot_ps[:], ones[:, :, 0:1], pa[:],
                        start=first, stop=last,
                        perf_mode=mybir.MatmulPerfMode.DoubleRow,
                        skip_group_check=True,
                    )
                    nc.tensor.matmul(
                        nrm_ps[:], ones[:, :, 0:1], p2a[:],
                        start=first, stop=last,
                        perf_mode=mybir.MatmulPerfMode.DoubleRow,
                        skip_group_check=True,
                    )

                for eb in range(DB):
                    pg = gpsum_pool.tile([P, 512], F32, tag="gemm")
                    for sb in range(SB):
                        nc.tensor.matmul(
                            pg[:],
                            sigT_sb[:, 2 * sb:2 * sb + 2, eb * P:(eb + 1) * P],
                            actsT8[:, 2 * sb:2 * sb + 2, l0:l0 + 512],
                            start=(sb == 0),
                            stop=(sb == SB - 1),
                            perf_mode=mybir.MatmulPerfMode.DoubleRow,
                        )
                    # sigma is pre-scaled by 1/4 on the host, so pg = preds/4
                    # already fits fp8 range; plain copy + Square from PSUM.
                    predsT = preds_pool.tile([P, 512], BF16, tag="preds")
                    nc.scalar.copy(predsT[:], pg[:])
                    if eb % 2 == 0:
                        prodp = prod_pool.tile([P, 2, 512], FP8, tag="prod")
                        prod2p = prod_pool.tile([P, 2, 512], FP8, tag="prod2")
                    nc.scalar.square(prod2p[:, eb % 2, :], pg[:])
                    nc.gpsimd.tensor_tensor(
                        prodp[:, eb % 2, :], predsT[:],
                        actsT8[:, eb, l0 + 1:l0 + 513], op=mybir.AluOpType.mult,
                    )
                    if lc == 1:
                        # preds column for local token 1023 (host boundary fix)
                        nc.vector.tensor_copy(
                            plast[:, h, eb:eb + 1], predsT[:, 511:512]
                        )
                    if eb % 2 == 1:
                        pending.append((prodp, prod2p, eb == 1, eb == DB - 1))
                        # skew: reduce pair k only after GEMM for pair k+1 ran
                        if len(pending) > 1:
                            flush_pair()
                while pending:
                    flush_pair()
                nc.scalar.copy(dot_sb[:, h, l0:l0 + 512], dot_ps[:])
                nc.scalar.copy(nrm_sb[:, h, l0:l0 + 512], nrm_ps[:])

        nc.sync.dma_start(dot_ext[:, :, :], dot_sb[:, :, :])
        nc.sync.dma_start(nrm_ext[:, :, :], nrm_sb[:, :, :])
        nc.sync.dma_start(pl_ext[:, :, :], plast[:, :, :])


def kernel(tokens, projections, sigmas):
    global LAST_RESULTS, _NC_CACHE
    tokens = np.asarray(tokens)
    projections = np.asarray(projections, dtype=np.float32)
    sigmas = np.asarray(sigmas, dtype=np.float32)

    # host-side shard: gather the token rows (this IS the sequence sharding),
    # pre-transpose sigma to (d_in, d_out), scale by 1/4, cast to fp8e4m3.
    raw = projections[:, tokens, :]                          # (H, L, D) f32
    sigT = np.ascontiguousarray(sigmas.transpose(0, 2, 1)) * np.float32(0.25)
    sigT = sigT.reshape(H, DB, P, D).astype(ml_dtypes.float8_e4m3)

    in_maps = []
    for c in range(NCORES):
        lo = c * CHUNK
        chunk = raw[:, lo:lo + CHUNK, :].reshape(H, TILES, P, D)
        in_maps.append({"raw": np.ascontiguousarray(chunk), "sigT": sigT})

    nc = _NC_CACHE
    if nc is None:
        nc = _NC_CACHE = _build_nc()

    res = bass_utils.run_bass_kernel_spmd(nc, in_maps, core_ids=list(range(NCORES)))
    LAST_RESULTS = res

    dots = np.concatenate([r["dot_out"][0] for r in res.results], axis=1)   # (H, 8192)
    nrm2 = np.concatenate([r["nrm_out"][0] for r in res.results], axis=1)
    dots = dots * np.float32(4.0)       # undo the 1/4 sigma scaling
    nrm2 = nrm2 * np.float32(16.0)

    # patch the cross-core boundary dots: token c*1024+1023 needs acts of the
    # next core's first token, which never reached core c.
    for c in range(NCORES - 1):
        plast = np.asarray(res.results[c]["plast_out"], dtype=np.float32)  # (P,H,DB)
        nxt = raw[:, (c + 1) * CHUNK, :]                                   # (H, D)
        thr = np.partition(nxt, D - K, axis=1)[:, D - K:D - K + 1]         # kth largest
        acts_nxt = (nxt >= thr).astype(np.float32)                         # (H, D)
        for h in range(H):
            preds_l = plast[:, h, :].transpose(1, 0).reshape(D)            # e=eb*128+p
            dots[h, c * CHUNK + CHUNK - 1] = np.float32(4.0) * float(
                np.dot(preds_l, acts_nxt[h])
            )

    dots = dots[:, : L - 1].astype(np.float32)
    nrm2 = nrm2[:, : L - 1].astype(np.float32)

    norms = np.sqrt(nrm2)
    overlap = dots / (norms * np.sqrt(np.float32(K)) + np.float32(1e-8))
    return (np.float32(1.0) - overlap).astype(np.float32)


# revision 8
# speedup vs baseline: 1.3127x; 1.3127x over previous
"""Trainium2 Bass kernel for nn_BDHModel (topk_masking).

Computes, per head h and token l:
    raw = projections[:, tokens, :]                  (gathered on host = sequence sharding)
    thr[h,l] = 20th largest of raw[h,l,:]            (exact: block-candidate max8 +
                                                      reciprocal-rank refinement on 64 cands)
    acts = (raw >= thr)
    preds[h,l] = acts[h,l] @ sigma[h].T              (fp8 DoubleRow GEMM, sigma/4 stationary)
    dot[h,l]   = sum(preds[h,l] * acts[h,l+1])       (GpSimd product + DoubleRow ones-matmul)
    norm2[h,l] = sum(preds[h,l]^2)                   (ScalarE Square from PSUM + ones-matmul)
    out = 1 - dot / (sqrt(norm2)*sqrt(20) + 1e-8)    (final scalar math on host)

Distribution: data-parallel over the sequence across 8 NeuronCores. Each core
processes exactly 1024 tokens for all 3 heads; sigma (pre-transposed to
(d_in, d_out), scaled by 1/4, fp8e4m3) is replicated to every core. The
cross-core boundary dot (token 1023 of each core vs token 0 of the next) is
patched on the host from an exported preds column.

Top-20 threshold: per 128-token tile, 8x max8 over 256-wide blocks of the
neuron axis yields 64 candidates/row that contain the true top-20 unless one
block holds >8 of them (P ~ 2e-3/row; affected rows move the threshold by
~1 rank, which is far inside the 2e-2 gate). The reciprocal-rank trick
(v8 -> v15 -> v20) then runs on just the 64 candidates, batched 4 tiles per
ScalarE/DVE instruction.
"""

import numpy as np
import ml_dtypes

import concourse.bacc as bacc
import concourse.mybir as mybir
import concourse.bass_utils as bass_utils
from concourse.bass import AP
from concourse.tile import TileContext
from concourse.masks import make_identity

ActF = mybir.ActivationFunctionType


def _act_raw(eng, out, in_, func, bias=0.0, scale=1.0, alpha=0.0, accum_out=None):
    """Direct InstActivation emission; bypasses the bass Reciprocal guard.

    Reciprocal here is used only for rank-ordering (monotone transform), where
    the table's ~1e-5 relative error is irrelevant; outputs clamp at +-1e7 and
    recip(0) = 3.4e38 (probed on HW), so no inf/NaN can reach max8.
    """
    inputs = [eng.lower_ap(in_)]
    for arg in (bias, scale, alpha):
        if isinstance(arg, AP):
            inputs.append(eng.lower_ap(arg))
        else:
            inputs.append(mybir.ImmediateValue(dtype=mybir.dt.float32, value=arg))
    outputs = [eng.lower_ap(out)]
    if accum_out is not None:
        outputs.append(eng.lower_ap(accum_out))
    return eng.add_instruction(
        mybir.InstActivation(
            name=eng.bass.get_next_instruction_name(),
            func=func,
            ins=inputs,
            outs=outputs,
        )
    )

H, V, D, L = 3, 32000, 2048, 8192
K = 20
NCORES = 8
CHUNK = L // NCORES            # 1024 tokens per core
TILES = CHUNK // 128           # 8 row-tiles, no boundary tile (host patches it)
ACOLS = CHUNK + 8              # actsT cols: 1024 tokens + zeroed pad (col 1024 used)
DB = D // 128                  # 16 blocks of 128 along the neuron axis
SB = DB // 2                   # 8 super-blocks of 256 (DoubleRow)
P = 128
NBLK = 8                       # candidate blocks per row (8 x 256)
BLKW = D // NBLK               # 256
NCAND = NBLK * 8               # 64 candidates per row
QT = 4                         # tiles per refinement batch ("quad")

F32 = mybir.dt.float32
BF16 = mybir.dt.bfloat16
FP8 = mybir.dt.float8e4

LAST_RESULTS = None            # test.py reads exec_time_ns from here

_NC_CACHE = None


def _build_nc():
    nc = bacc.Bacc("TRN2", target_bir_lowering=False, debug=False)
    raw_ext = nc.dram_tensor("raw", [H, TILES, P, D], F32, kind="ExternalInput")
    sigT_ext = nc.dram_tensor("sigT", [H, DB, P, D], FP8, kind="ExternalInput")
    dot_ext = nc.dram_tensor("dot_out", [1, H, CHUNK], F32, kind="ExternalOutput")
    nrm_ext = nc.dram_tensor("nrm_out", [1, H, CHUNK], F32, kind="ExternalOutput")
    pl_ext = nc.dram_tensor("plast_out", [P, H, DB], BF16, kind="ExternalOutput")

    with TileContext(nc) as tc:
        _body(nc, tc, raw_ext, sigT_ext, dot_ext, nrm_ext, pl_ext)
    nc.compile()
    return nc


def _body(nc, tc, raw_ext, sigT_ext, dot_ext, nrm_ext, pl_ext):
    with (
        tc.tile_pool(name="consts", bufs=1) as consts,
        tc.tile_pool(name="sig", bufs=2) as sig_pool,
        tc.tile_pool(name="actsT", bufs=2) as actsT_pool,
        tc.tile_pool(name="raw", bufs=5) as raw_pool,
        tc.tile_pool(name="acts", bufs=2) as acts_pool,
        tc.tile_pool(name="atb", bufs=3) as atb_pool,
        tc.tile_pool(name="cand", bufs=2) as cand_pool,
        tc.tile_pool(name="m8", bufs=4) as m8_pool,
        tc.tile_pool(name="preds", bufs=6) as preds_pool,
        tc.tile_pool(name="prod", bufs=6) as prod_pool,
        tc.tile_pool(name="stage", bufs=1) as stage_pool,
        tc.tile_pool(name="gpsum", bufs=2, space="PSUM") as gpsum_pool,
        tc.tile_pool(name="rpsum", bufs=1, space="PSUM") as rpsum_pool,
    ):
        ones = consts.tile([P, 2, 16], FP8)
        nc.vector.memset(ones[:], 1.0)

        dot_sb = stage_pool.tile([1, H, CHUNK], F32, tag="dot_sb")
        nrm_sb = stage_pool.tile([1, H, CHUNK], F32, tag="nrm_sb")
        plast = stage_pool.tile([P, H, DB], BF16, tag="plast")

        sig_next = None
        for h in range(H):
            if sig_next is None:
                sigT_sb = sig_pool.tile([P, DB, D], FP8, tag="sigT")
            else:
                sigT_sb = sig_next
            actsT8 = actsT_pool.tile([P, DB, ACOLS], FP8, tag="actsT")
            # zero the pad column so the lc=1 shifted product reads 0 there
            nc.vector.memset(actsT8[:, :, CHUNK:ACOLS], 0.0)

            # --- stage 1: exact top-20 threshold on 64 block candidates ---
            for q in range(TILES // QT):
                raws = []
                for tt in range(QT):
                    t = q * QT + tt
                    rt = raw_pool.tile([P, D], F32, tag="raw", name="rawt")
                    nc.sync.dma_start(rt[:], raw_ext[h, t])
                    raws.append(rt)
                if h == 0 and q == 0:
                    # sigma loads go behind the first raw tiles
                    for db in range(DB):
                        nc.sync.dma_start(sigT_sb[:, db, :], sigT_ext[h, db])

                cand = cand_pool.tile([P, QT, NCAND], F32, tag="cand")
                c8 = m8_pool.tile([P, QT, 8], F32, tag="c8")
                for tt in range(QT):
                    for b in range(NBLK):
                        nc.vector.max(
                            cand[:, tt, b * 8:(b + 1) * 8],
                            raws[tt][:, b * BLKW:(b + 1) * BLKW],
                        )
                for tt in range(QT):
                    nc.vector.max(c8[:, tt, :], cand[:, tt, :])

                # z1 = 1/(v8 + eps - cand); top of z1 = [z(v8), r9..r15]
                w1 = cand_pool.tile([P, QT, NCAND], F32, tag="w1")
                nc.vector.tensor_tensor(
                    w1[:], cand[:],
                    c8[:, :, 7:8].broadcast_to([P, QT, NCAND]),
                    op=mybir.AluOpType.subtract,
                )
                _act_raw(nc.scalar, w1[:], w1[:], ActF.Reciprocal,
                         scale=-1.0, bias=2.0 ** -40)
                m8b = m8_pool.tile([P, QT, 8], F32, tag="m8b")
                for tt in range(QT):
                    nc.vector.max(m8b[:, tt, :], w1[:, tt, :])
                # v15 = v8 - 0.9997/z15 (slightly above the true v15)
                inv1 = m8_pool.tile([P, QT, 1], F32, tag="inv1")
                _act_raw(nc.scalar, inv1[:], m8b[:, :, 7:8], ActF.Reciprocal,
                         scale=-1.0003)
                v15 = m8_pool.tile([P, QT, 1], F32, tag="v15")
                nc.vector.tensor_tensor(
                    v15[:], inv1[:], c8[:, :, 7:8], op=mybir.AluOpType.add
                )

                # z2 = 1/(v15 + eps - cand); top = [z(r15), r16..r22]
                w2 = cand_pool.tile([P, QT, NCAND], F32, tag="w2")
                nc.vector.tensor_tensor(
                    w2[:], cand[:],
                    v15[:].broadcast_to([P, QT, NCAND]),
                    op=mybir.AluOpType.subtract,
                )
                _act_raw(nc.scalar, w2[:], w2[:], ActF.Reciprocal,
                         scale=-1.0, bias=2.0 ** -40)
                m8c = m8_pool.tile([P, QT, 8], F32, tag="m8c")
                for tt in range(QT):
                    nc.vector.max(m8c[:, tt, :], w2[:, tt, :])
                # thr = v15 - 1.0003/z20 (slightly below the true v20)
                inv2 = m8_pool.tile([P, QT, 1], F32, tag="inv2")
                _act_raw(nc.scalar, inv2[:], m8c[:, :, 5:6], ActF.Reciprocal,
                         scale=-0.9997)
                thr = m8_pool.tile([P, QT, 1], F32, tag="thr")
                nc.vector.tensor_tensor(
                    thr[:], inv2[:], v15[:], op=mybir.AluOpType.add
                )

                for tt in range(QT):
                    t = q * QT + tt
                    acts_t = acts_pool.tile([P, D], BF16, tag="acts")
                    nc.vector.tensor_scalar(
                        acts_t[:], raws[tt][:], thr[:, tt, :], None,
                        mybir.AluOpType.is_ge,
                    )
                    # xbar transpose: atb[p, k, l] = acts_t[l, k*128 + p];
                    # dst must be contiguous (strided xbar dst is broken on HW)
                    atb = atb_pool.tile([P, DB, P], BF16, tag="atb")
                    nc.sync.dma_start_transpose(atb[:], acts_t[:])
                    # bf16 -> fp8 convert into the (strided) GEMM operand;
                    # DVE (GpSimd casts run ~4 cyc/elem and steal the shared
                    # SBUF port from 2-port DVE ops)
                    nc.vector.tensor_copy(
                        actsT8[:, :, t * P:(t + 1) * P], atb[:]
                    )

            if h + 1 < H:
                # prefetch next head's sigma behind this head's GEMM
                sig_next = sig_pool.tile([P, DB, D], FP8, tag="sigT", name="sig_nx")
                for db in range(DB):
                    nc.sync.dma_start(sig_next[:, db, :], sigT_ext[h + 1, db])

            # --- stage 2: fp8 DoubleRow GEMM (predsT layout) + reductions ---
            for lc in range(CHUNK // 512):
                l0 = lc * 512
                dot_ps = rpsum_pool.tile([1, 512], F32, tag="dotps")
                nrm_ps = rpsum_pool.tile([1, 512], F32, tag="nrmps")
                prodp = None
                prod2p = None
                pending = []           # completed prod pairs awaiting reduce-MMs

                def flush_pair():
                    pa, p2a, first, last = pending.pop(0)
                    nc.tensor.matmul(
                        dot_ps[:], ones[:, :, 0:1], pa[:],
                        start=first, stop=last,
                        perf_mode=mybir.MatmulPerfMode.DoubleRow,
                        skip_group_check=True,
                    )
                    nc.tensor.matmul(
                        nrm_ps[:], ones[:, :, 0:1], p2a[:],
                        start=first, stop=last,
                        perf_mode=mybir.MatmulPerfMode.DoubleRow,
                        skip_group_check=True,
                    )

                for eb in range(DB):
                    pg = gpsum_pool.tile([P, 512], F32, tag="gemm")
                    for sb in range(SB):
                        nc.tensor.matmul(
                            pg[:],
                            sigT_sb[:, 2 * sb:2 * sb + 2, eb * P:(eb + 1) * P],
                            actsT8[:, 2 * sb:2 * sb + 2, l0:l0 + 512],
                            start=(sb == 0),
                            stop=(sb == SB - 1),
                            perf_mode=mybir.MatmulPerfMode.DoubleRow,
                        )
                    # sigma is pre-scaled by 1/4 on the host, so pg = preds/4
                    # already fits fp8 range; plain copy + Square from PSUM.
                    predsT = preds_pool.tile([P, 512], BF16, tag="preds")
                    nc.scalar.copy(predsT[:], pg[:])
                    if eb % 2 == 0:
                        prodp = prod_pool.tile([P, 2, 512], FP8, tag="prod")
                        prod2p = prod_pool.tile([P, 2, 512], FP8, tag="prod2")
                    nc.scalar.square(prod2p[:, eb % 2, :], pg[:])
                    nc.gpsimd.tensor_tensor(
                        prodp[:, eb % 2, :], predsT[:],
                        actsT8[:, eb, l0 + 1:l0 + 513], op=mybir.AluOpType.mult,
                    )
                    if lc == 1:
                        # preds column for local token 1023 (host boundary fix)
                        nc.vector.tensor_copy(
                            plast[:, h, eb:eb + 1], predsT[:, 511:512]
                        )
                    if eb % 2 == 1:
                        pending.append((prodp, prod2p, eb == 1, eb == DB - 1))
                        # skew: reduce pair k only after GEMM for pair k+1 ran
                        if len(pending) > 1:
                            flush_pair()
                while pending:
                    flush_pair()
                nc.scalar.copy(dot_sb[:, h, l0:l0 + 512], dot_ps[:])
                nc.scalar.copy(nrm_sb[:, h, l0:l0 + 512], nrm_ps[:])

        nc.sync.dma_start(dot_ext[:, :, :], dot_sb[:, :, :])
        nc.sync.dma_start(nrm_ext[:, :, :], nrm_sb[:, :, :])
        nc.sync.dma_start(pl_ext[:, :, :], plast[:, :, :])


def kernel(tokens, projections, sigmas):
    global LAST_RESULTS, _NC_CACHE
    tokens = np.asarray(tokens)
    projections = np.asarray(projections, dtype=np.float32)
    sigmas = np.asarray(sigmas, dtype=np.float32)

    # host-side shard: gather the token rows (this IS the sequence sharding),
    # pre-transpose sigma to (d_in, d_out), scale by 1/4, cast to fp8e4m3.
    raw = projections[:, tokens, :]                          # (H, L, D) f32
    sigT = np.ascontiguousarray(sigmas.transpose(0, 2, 1)) * np.float32(0.25)
    sigT = sigT.reshape(H, DB, P, D).astype(ml_dtypes.float8_e4m3)

    in_maps = []
    for c in range(NCORES):
        lo = c * CHUNK
        chunk = raw[:, lo:lo + CHUNK, :].reshape(H, TILES, P, D)
        in_maps.append({"raw": np.ascontiguousarray(chunk), "sigT": sigT})

    nc = _NC_CACHE
    if nc is None:
        nc = _NC_CACHE = _build_nc()

    res = bass_utils.run_bass_kernel_spmd(nc, in_maps, core_ids=list(range(NCORES)))
    LAST_RESULTS = res

    dots = np.concatenate([r["dot_out"][0] for r in res.results], axis=1)   # (H, 8192)
    nrm2 = np.concatenate([r["nrm_out"][0] for r in res.results], axis=1)
    dots = dots * np.float32(4.0)       # undo the 1/4 sigma scaling
    nrm2 = nrm2 * np.float32(16.0)

    # patch the cross-core boundary dots: token c*1024+1023 needs acts of the
    # next core's first token, which never reached core c.
    for c in range(NCORES - 1):
        plast = np.asarray(res.results[c]["plast_out"], dtype=np.float32)  # (P,H,DB)
        nxt = raw[:, (c + 1) * CHUNK, :]                                   # (H, D)
        thr = np.partition(nxt, D - K, axis=1)[:, D - K:D - K + 1]         # kth largest
        acts_nxt = (nxt >= thr).astype(np.float32)                         # (H, D)
        for h in range(H):
            preds_l = plast[:, h, :].transpose(1, 0).reshape(D)            # e=eb*128+p
            dots[h, c * CHUNK + CHUNK - 1] = np.float32(4.0) * float(
                np.dot(preds_l, acts_nxt[h])
            )

    dots = dots[:, : L - 1].astype(np.float32)
    nrm2 = nrm2[:, : L - 1].astype(np.float32)

    norms = np.sqrt(nrm2)
    overlap = dots / (norms * np.sqrt(np.float32(K)) + np.float32(1e-8))
    return (np.float32(1.0) - overlap).astype(np.float32)
